# revision 19
# baseline (speedup 1.0000x reference)
"""GCN 3-layer block on 8 Trainium2 NeuronCores.

Strategy (data-parallel over the 32 graph replicas, 4 graphs/core):
  - The GCN aggregation  agg = A_hat @ h  (A_hat = D^-1/2 (Adj + 2I) D^-1/2,
    E=16K edges over L=2048 nodes) is a DENSE bf16 matmul on the
    TensorEngine. A_hat is factored as  Dl @ (M + 2I) @ Dr  with M the
    integer edge-count matrix (shipped bf16-exact, 16 chunks across two
    DMA rings in consumption order); Dr (per-src-node scale) is folded
    into the producers of every aggregation input (host-side for h0, the
    per-tile W2/W3 PSUM drains otherwise); Dl (per-dst-node scale) is
    folded into the aggregation drains via a partition-replicated dis-row
    tile (scalar_tensor_tensor with fused stats accumulation).
  - Layer ordering minimizes aggregation width AND avoids every transpose:
      L1: agg@64 (graph-paired var2, LC->CL) -> W1 (CL->CL, interleaved
          per dst-block so it fills AT-DMA gaps and stats finish early).
      L2: W2 data-stationary (CL->LC) -> agg@128 (var2, LC->CL).
      L3: W3 data-stationary (CL->LC, graph-paired) -> agg@64 (var2 -> CL).
  - BatchNorm statistics: per-channel sums fused into the drain
    accumulators (DVE); sum-of-squares via ScalarE Square passes reading
    the same PSUM chunk in parallel with the drain. The tiny [128,2]
    packs are AllReduce'd across the 8 cores. The first collective on the
    CC stream pays a one-time ~25-42us ncfw init barrier that starts only
    when the LAST core rings (~12us core-start skew), so a warmup
    AllReduce on an uninitialized scratch tensor is issued as the very
    first GpSimd instruction -- the barrier + warmup run concurrently
    with the input DMAs and all of layer 1, and the layer-1 stats
    exchange starts as soon as the stream frees (~70us).
  - Stall mitigation: paced junk-matmul chains (DVE self-dependency
    pacer, one PE matmul per ~0.7us link) keep the PE clock warm through
    each AllReduce wait; the post-stats ReLU for the first graph runs in
    4-tile quarters alternating ScalarE/DVE so the next W stage restarts
    the PE within ~1us of the scale/shift landing.
  - HAM/table warmup: junk matmuls on a zeroed tile run while the input
    DMAs land (PE at 2.4GHz from the first real matmul), and a dummy Sqrt
    warms the ScalarE activation-table set off the critical path.
  - Tail: BN3+ReLU in halves split ScalarE/DVE; all output DMAs issue on
    the SP ring so they never serialize behind the ReLUs on ACT.
"""

import numpy as np
import ml_dtypes

import concourse.bass as bass
import concourse.bacc as bacc
import concourse.mybir as mybir
import concourse.tile as tile
from concourse.bass_utils import run_bass_kernel_spmd

BF16 = ml_dtypes.bfloat16

# Problem constants (nn_GCN1dBlock: x [4,8,64,2048], E=16384)
B, NREP, C0, L = 4, 8, 64, 2048
G_TOTAL = B * NREP          # 32 graphs
N_CORES = 8
G = G_TOTAL // N_CORES      # 4 graphs per core
NT = L // 128               # 16 node tiles
N_ROWS = G_TOTAL * L        # BN reduction length (global)
EPS = 1e-5
NQ = 4                       # SWDGE queues used for the stats exchange
FP32 = mybir.dt.float32
BF = mybir.dt.bfloat16
U8 = mybir.dt.uint8
ADD = mybir.AluOpType.add
MUL = mybir.AluOpType.mult
SUB = mybir.AluOpType.subtract
SQUARE = mybir.ActivationFunctionType.Square
RELU = mybir.ActivationFunctionType.Relu
SQRT = mybir.ActivationFunctionType.Sqrt
IDENT = mybir.ActivationFunctionType.Identity


def build_program():
    nc = bacc.Bacc(None, target_bir_lowering=False, num_devices=N_CORES)

    # I/O --------------------------------------------------------------
    # M+2I packed [jb, kb, p, mj, kr, q] as uint8: 16 DRAM chunks of
    # (4 dst-tiles x 4 src-tiles); chunk (jb, kb) covers dst tiles
    # m=4jb+mj, src tiles k=4kb+kr.
    at_dram = nc.dram_tensor("at", [4, 4, 128, 4, 4, 128], BF, kind="ExternalInput")
    # h0 packed per graph-pair: [gp, p, k, c2], PRE-SCALED by dis[node]
    h0_dram = nc.dram_tensor("h0", [2, 128, NT, 128], BF, kind="ExternalInput")
    w1_dram = nc.dram_tensor("w1", [128, 128], BF, kind="ExternalInput")
    w2_dram = nc.dram_tensor("w2", [128, 128], BF, kind="ExternalInput")
    w3_dram = nc.dram_tensor("w3", [128, 64], BF, kind="ExternalInput")
    # bn params: columns = [g1, be1, g2, be2, g3, be3] (g3/be3 in rows 0:64)
    bn_dram = nc.dram_tensor("bn", [128, 6], FP32, kind="ExternalInput")
    # fold[p, c] = (p % 64 == c): folds pair-stacked stats; dup = fold^T
    fold_dram = nc.dram_tensor("fold", [128, 64], FP32, kind="ExternalInput")
    dup_dram = nc.dram_tensor("dup", [64, 128], FP32, kind="ExternalInput")
    # dis by node: disr = [1, L] (row for partition-broadcast),
    # disc = [128, NT] (disc[p, k] = dis[128k+p], per-partition scales)
    disr_dram = nc.dram_tensor("disr", [1, L], FP32, kind="ExternalInput")
    disc_dram = nc.dram_tensor("disc", [128, NT], FP32, kind="ExternalInput")
    out_dram = nc.dram_tensor("out", [G, 64, L], FP32, kind="ExternalOutput")

    warm_in = nc.dram_tensor("warm_in", [128, 2], FP32)
    warm_out = nc.dram_tensor("warm_out", [128, 2], FP32, addr_space="Shared")
    stats_in = [nc.dram_tensor(f"stats_in{i}", [128, 2], FP32) for i in range(3)]
    stats_out = [
        nc.dram_tensor(f"stats_out{i}", [128, 2], FP32, addr_space="Shared")
        for i in range(3)
    ]

    with tile.TileContext(nc) as tc:
        with (
            tc.tile_pool(name="const", bufs=1) as constp,
            tc.tile_pool(name="work", bufs=1) as work,
            tc.tile_pool(name="outp", bufs=2) as outp,
            tc.tile_pool(name="stat", bufs=1) as statp,
            tc.tile_pool(name="junk", bufs=2) as junkp,
            tc.tile_pool(name="pa", bufs=3, space=bass.MemorySpace.PSUM) as pa,
            tc.tile_pool(name="pw", bufs=2, space=bass.MemorySpace.PSUM) as pw,
            tc.tile_pool(name="pw3", bufs=2, space=bass.MemorySpace.PSUM) as pw3,
            tc.tile_pool(name="pj", bufs=1, space=bass.MemorySpace.PSUM) as pj,
        ):
            # ---- warmup collective: doorbell ASAP -----------------------
            # Initializes the collectives stack (ncfw barrier) in the
            # background; nothing on the critical path waits for it.
            # warm_in is read UNINITIALIZED (values irrelevant) so the
            # doorbell has no DMA dependency and fires immediately.
            nc.gpsimd.collective_compute(
                "AllReduce", ADD,
                replica_groups=[list(range(N_CORES))],
                ins=[warm_in[:]],
                outs=[warm_out[:]],
            )
            # junk tile for the HAM-warmup matmuls (also on GpSimd queue)
            jw = constp.tile([128, 512], BF, tag="jw")
            nc.gpsimd.memset(jw[:], 0.0)
            eps_t = constp.tile([128, 1], FP32, tag="eps")
            nc.gpsimd.memset(eps_t[:], EPS)

            packs = [
                statp.tile([128, 2], FP32, tag=f"pack{l}", name=f"pack{l}")
                for l in range(3)
            ]
            nc.vector.memset(packs[2][:], 0.0)  # rows 64:127 stay zero (L3)

            # ---- input DMAs ------------------------------------------
            # Two rings (ACT + SP), chunks ordered by when L1 consumes them.
            h0t = [
                constp.tile([128, NT, 128], BF, tag=f"h0_{gp}", name=f"h0_{gp}")
                for gp in range(2)
            ]
            w1 = constp.tile([128, 128], BF, tag="w1")  # W1 duplicated rows
            w2 = constp.tile([128, 128], BF, tag="w2")
            w3 = constp.tile([128, 64], BF, tag="w3")
            bn = constp.tile([128, 6], FP32, tag="bn")
            foldc = constp.tile([128, 64], FP32, tag="fold")
            dupc = constp.tile([64, 128], FP32, tag="dup")
            disr1 = constp.tile([1, L], FP32, tag="disr1")
            disc = constp.tile([128, NT], FP32, tag="disc")

            # M chunk tiles (bf16, DMA'd directly; counts are exact ints)
            at4 = []
            for j in range(4):
                row = []
                for kb in range(4):
                    row.append(
                        constp.tile([128, 4, 4, 128], BF, tag=f"at{j}_{kb}",
                                    name=f"at{j}_{kb}")
                    )
                at4.append(row)

            def at_dma(j, kb, q):
                q.dma_start(at4[j][kb][:], at_dram[j, kb])

            # ACT ring: h0 gp0, small consts, then odd-kb M chunks.
            nc.scalar.dma_start(h0t[0][:], h0_dram[0])
            nc.scalar.dma_start(w1[:], w1_dram[:])
            nc.scalar.dma_start(w2[:], w2_dram[:])
            nc.scalar.dma_start(w3[:], w3_dram[:])
            nc.scalar.dma_start(bn[:], bn_dram[:])
            nc.scalar.dma_start(foldc[:], fold_dram[:])
            nc.scalar.dma_start(dupc[:], dup_dram[:])
            nc.scalar.dma_start(disr1[:], disr_dram[:])
            nc.scalar.dma_start(disc[:], disc_dram[:])
            # SP ring: first M chunk, h0 gp1, then the rest interleaved.
            at_dma(0, 0, nc.sync)
            nc.sync.dma_start(h0t[1][:], h0_dram[1])
            at_dma(0, 1, nc.scalar)
            at_dma(0, 2, nc.sync)
            at_dma(0, 3, nc.scalar)

            # dis-row replicated across partitions: [128, L] fp32
            disrow = constp.tile([128, L], FP32, tag="disrow")
            nc.gpsimd.partition_broadcast(disrow[:], disr1[:])

            # Warm the ScalarE Sqrt table set now, off the critical path.
            rsqw = statp.tile([128, 1], FP32, tag="rsqw")
            nc.scalar.activation(rsqw[:], eps_t[:], SQRT)

            # ---- HAM warmup: junk matmuls while input DMAs land ------
            pjt = pj.tile([128, 512], FP32, tag="pj")
            for _ in range(16):
                nc.tensor.matmul(pjt[:], jw[:, 0:128], jw[:], start=True, stop=True)


            def at_rhs(jb, k):
                return at4[jb][k // 4][:, :, k % 4, :]

            jpA = statp.tile([128, 2048], BF, tag="jpA")
            jpB = statp.tile([128, 2048], BF, tag="jpB")

            def warm_keep(layer, links):
                """Paced junk matmuls anchored at the stats stall: a DVE
                self-dependency chain (~0.7us/link) paces one PE matmul per
                link so HAM never sees a >3.4us idle window during the
                AllReduce wait."""
                nc.vector.tensor_scalar(jpA[:, 0:2], packs[layer][:], 1.0,
                                        None, MUL)
                cur, nxt = jpA, jpB
                for _ in range(links):
                    nc.vector.tensor_scalar(nxt[:], cur[:], 1.0, None, MUL)
                    nc.tensor.matmul(pjt[:], jw[:, 0:128], nxt[:, 0:512],
                                     start=True, stop=True)
                    cur, nxt = nxt, cur

            def stats_chain(layer, cpart):
                """AllReduce the pack and build scale/shift [cpart,3]."""
                nc.sync.dma_start(stats_in[layer][:], packs[layer][:])
                nc.gpsimd.collective_compute(
                    "AllReduce", ADD,
                    replica_groups=[list(range(N_CORES))],
                    ins=[stats_in[layer][:]],
                    outs=[stats_out[layer][:]],
                )
                red = statp.tile([128, 2], FP32, tag=f"red{layer}", name="red")
                nc.sync.dma_start(red[:], stats_out[layer][:])

                mom = statp.tile([cpart, 4], FP32, tag=f"mom{layer}")
                # mom cols: 0=mean, 1=E[x^2], 2=var, 3=sqrt(var+eps)
                nc.vector.tensor_scalar(mom[:, 0:2], red[:cpart, 0:2],
                                        1.0 / N_ROWS, None, MUL)
                nc.vector.tensor_tensor(mom[:, 2:3], mom[:, 0:1], mom[:, 0:1], MUL)
                nc.vector.tensor_tensor(mom[:, 2:3], mom[:, 1:2], mom[:, 2:3], SUB)
                nc.scalar.activation(mom[:, 3:4], mom[:, 2:3], SQRT,
                                     bias=eps_t[:cpart, :])
                ss = statp.tile([cpart, 3], FP32, tag=f"ss{layer}")
                # ss cols: 0=rsqrt, 1=scale, 2=shift
                nc.vector.reciprocal(ss[:, 0:1], mom[:, 3:4])
                nc.vector.tensor_tensor(
                    ss[:, 1:2], ss[:, 0:1], bn[:cpart, 2 * layer : 2 * layer + 1],
                    MUL,
                )
                nc.vector.tensor_tensor(ss[:, 2:3], mom[:, 0:1], ss[:, 1:2], MUL)
                nc.vector.tensor_tensor(
                    ss[:, 2:3], bn[:cpart, 2 * layer + 1 : 2 * layer + 2],
                    ss[:, 2:3], SUB,
                )
                return ss

            # ================= Layer 1 ================================
            # agg1 (var2, graph-paired): lhsT = h0 chunk [128src, 2x64ch]
            # (h0 pre-scaled by Dr on the host), rhs = M chunk -> out CL;
            # drains apply Dl (dis-row) via scalar_tensor_tensor. W1
            # interleaved per dst-block; its drains carry the BN1 sums and
            # its PSUM is Square'd for the sumsq (both fully scaled).
            # Next jb's chunk DMAs + conversions are issued per iteration
            # so engine queues never wait on not-yet-needed chunks.
            agg1_cl = work.tile([128, 2, NT, 128], BF, tag="agg1")
            h1pre = work.tile([128, G, NT, 128], BF, tag="hpre")
            acc1_s = statp.tile([128, 16], FP32, tag="acc1s")
            acc1_q = statp.tile([128, 16], FP32, tag="acc1q")
            for jb in range(4):
                if jb < 3:
                    nj = jb + 1
                    at_dma(nj, 0, nc.sync)
                    at_dma(nj, 1, nc.scalar)
                    at_dma(nj, 2, nc.sync)
                    at_dma(nj, 3, nc.scalar)
                dr = disrow[:, 512 * jb : 512 * jb + 512]
                for gp in range(2):
                    ps = pa.tile([128, 512], FP32, tag="pa")
                    for k in range(NT):
                        nc.tensor.matmul(
                            ps[:],
                            h0t[gp][:, k, :],
                            at_rhs(jb, k),
                            start=(k == 0), stop=(k == NT - 1),
                        )
                    nc.vector.scalar_tensor_tensor(
                        agg1_cl[:, gp, 4 * jb : 4 * jb + 4, :], ps[:], 1.0, dr,
                        MUL, MUL,
                    )
                for g in range(G):
                    psw = pw.tile([128, 512], FP32, tag="pw")
                    nc.tensor.matmul(
                        psw[:],
                        w1[64 * (g % 2) : 64 * (g % 2) + 64, :],
                        agg1_cl[64 * (g % 2) : 64 * (g % 2) + 64,
                                g // 2, 4 * jb : 4 * jb + 4, :],
                        start=True, stop=True,
                    )
                    col = 4 * g + jb
                    nc.vector.tensor_scalar(
                        h1pre[:, g, 4 * jb : 4 * jb + 4, :], psw[:], 0.0, None,
                        ADD, ADD, accum_out=acc1_s[:, col : col + 1],
                    )
                    sqj = junkp.tile([128, 512], BF, tag="sqj")
                    if g == G - 1 and jb == 3:
                        # last chunk's sumsq on DVE to shorten the tail
                        sl = h1pre[:, g, 4 * jb : 4 * jb + 4, :]
                        nc.vector.scalar_tensor_tensor(
                            sqj[:], sl, 1.0, sl, MUL, MUL,
                            accum_out=acc1_q[:, col : col + 1],
                        )
                    else:
                        nc.scalar.activation(
                            sqj[:], psw[:], SQUARE,
                            accum_out=acc1_q[:, col : col + 1],
                        )

            nc.vector.tensor_reduce(packs[0][:, 0:1], acc1_s[:, :16],
                                    axis=mybir.AxisListType.X, op=ADD)
            nc.vector.tensor_reduce(packs[0][:, 1:2], acc1_q[:, :16],
                                    axis=mybir.AxisListType.X, op=ADD)
            warm_keep(0, 24)
            ss1 = stats_chain(0, 128)

            # ================= Layer 2 (W-first, no transposes) ========
            # BN1+relu per graph (g0 split ScalarE/DVE so the PE restarts
            # fastest after the stats wait), then W2 data-stationary with
            # per-tile Dr-scaled drains emitting LC, then agg2 (var2) with
            # Dl + BN2 sums fused into the drains.
            h1_cl = work.tile([128, G, NT, 128], BF, tag="h_cl")
            h1w = work.tile([128, G, NT, 128], BF, tag="h_w")
            agg2_cl = work.tile([128, G, NT, 128], BF, tag="hpre")
            acc2_s = statp.tile([128, 16], FP32, tag="acc2s")
            acc2_q = statp.tile([128, 16], FP32, tag="acc2q")

            def bn_relu(dst, src, ss, g, split):
                """dst[:,g] = relu(ss.scale * src[:,g] + ss.shift)."""
                if split:
                    # quarters, SE/DVE alternating: the first 4-tile block
                    # (all the first W block needs) is ready in ~0.7us
                    for q in range(4):
                        sl_d = dst[:, g, 4 * q : 4 * q + 4, :]
                        sl_s = src[:, g, 4 * q : 4 * q + 4, :]
                        if q % 2 == 0:
                            nc.scalar.activation(
                                sl_d, sl_s,
                                RELU, bias=ss[:, 2:3], scale=ss[:, 1:2],
                            )
                        else:
                            nc.vector.tensor_scalar(
                                sl_d, sl_s, ss[:, 1:2], ss[:, 2:3], MUL, ADD,
                            )
                            nc.vector.tensor_scalar_max(sl_d, sl_d, 0.0)
                else:
                    for h in range(2):
                        nc.scalar.activation(
                            dst[:, g, 8 * h : 8 * h + 8, :],
                            src[:, g, 8 * h : 8 * h + 8, :],
                            RELU, bias=ss[:, 2:3], scale=ss[:, 1:2],
                        )

            for g in range(G):
                bn_relu(h1_cl, h1pre, ss1, g, split=(g == 0))
                for jb in range(4):
                    psj = pw.tile([128, 512], FP32, tag="pw")
                    for j in range(4):
                        nc.tensor.matmul(
                            psj[:, 128 * j : 128 * j + 128],
                            h1_cl[:, g, 4 * jb + j, :], w2[:],
                            start=True, stop=True,
                        )
                    for j in range(4):
                        nc.vector.tensor_scalar(
                            h1w[:, g, 4 * jb + j, :],
                            psj[:, 128 * j : 128 * j + 128],
                            disc[:, 4 * jb + j : 4 * jb + j + 1], None, MUL,
                        )
                for jb in range(4):
                    ps = pa.tile([128, 512], FP32, tag="pa")
                    for k in range(NT):
                        nc.tensor.matmul(
                            ps[:],
                            h1w[:, g, k, :],
                            at_rhs(jb, k),
                            start=(k == 0), stop=(k == NT - 1),
                        )
                    col = 4 * g + jb
                    dst = agg2_cl[:, g, 4 * jb : 4 * jb + 4, :]
                    nc.vector.scalar_tensor_tensor(
                        dst, ps[:], 1.0, disrow[:, 512 * jb : 512 * jb + 512],
                        MUL, MUL, accum_out=acc2_s[:, col : col + 1],
                    )
                    sqj = junkp.tile([128, 512], BF, tag="sqj")
                    if g == G - 1 and jb == 3:
                        nc.vector.scalar_tensor_tensor(
                            sqj[:], dst, 1.0, dst, MUL, MUL,
                            accum_out=acc2_q[:, col : col + 1],
                        )
                    else:
                        nc.scalar.activation(
                            sqj[:], dst, SQUARE,
                            accum_out=acc2_q[:, col : col + 1],
                        )

            nc.vector.tensor_reduce(packs[1][:, 0:1], acc2_s[:, :16],
                                    axis=mybir.AxisListType.X, op=ADD)
            nc.vector.tensor_reduce(packs[1][:, 1:2], acc2_q[:, :16],
                                    axis=mybir.AxisListType.X, op=ADD)
            warm_keep(1, 16)
            ss2 = stats_chain(1, 128)

            # ================= Layer 3 ================================
            # BN2+relu (g0 split engines) then W3 data-stationary per graph
            # with per-tile Dr-scaled drains, emitting LC pair-packed;
            # agg3 (var2, graph-paired 2x64ch) with Dl + stats fused;
            # pair-stacked stats folded via tiny PE matmuls.
            h2_cl = work.tile([128, G, NT, 128], BF, tag="h_cl")
            # h2w: [p=node, pair, k, (gi*64 + c)] -- pair channels contiguous
            h2w = work.tile([128, 2, NT, 128], BF, tag="agg1")
            for g in range(G):
                bn_relu(h2_cl, agg2_cl, ss2, g, split=(g == 0))
                c0 = 64 * (g % 2)
                for jb in range(4):
                    psj = pw3.tile([128, 4, 64], FP32, tag="pw3")
                    for j in range(4):
                        nc.tensor.matmul(
                            psj[:, j, :], h2_cl[:, g, 4 * jb + j, :], w3[:],
                            start=True, stop=True,
                        )
                    for j in range(4):
                        nc.vector.tensor_scalar(
                            h2w[:, g // 2, 4 * jb + j, c0 : c0 + 64],
                            psj[:, j, :],
                            disc[:, 4 * jb + j : 4 * jb + j + 1], None, MUL,
                        )

            agg3_cl = work.tile([128, 2, NT, 128], BF, tag="agg3")
            acc3_s = statp.tile([128, 8], FP32, tag="acc3s")
            acc3_q = statp.tile([128, 8], FP32, tag="acc3q")
            for p in range(2):
                for jb in range(4):
                    ps = pa.tile([128, 512], FP32, tag="pa")
                    for k in range(NT):
                        nc.tensor.matmul(
                            ps[:],
                            h2w[:, p, k, :],
                            at_rhs(jb, k),
                            start=(k == 0), stop=(k == NT - 1),
                        )
                    col = 4 * p + jb
                    dst = agg3_cl[:, p, 4 * jb : 4 * jb + 4, :]
                    nc.vector.scalar_tensor_tensor(
                        dst, ps[:], 1.0, disrow[:, 512 * jb : 512 * jb + 512],
                        MUL, MUL, accum_out=acc3_s[:, col : col + 1],
                    )
                    sqj = junkp.tile([128, 512], BF, tag="sqj")
                    if p == 1 and jb == 3:
                        nc.vector.scalar_tensor_tensor(
                            sqj[:], dst, 1.0, dst, MUL, MUL,
                            accum_out=acc3_q[:, col : col + 1],
                        )
                    else:
                        nc.scalar.activation(
                            sqj[:], dst, SQUARE,
                            accum_out=acc3_q[:, col : col + 1],
                        )

            # fold pair-stacked stats [128,2] -> rows 0:64 via PE f32 matmul
            pack3r = statp.tile([128, 2], FP32, tag="pack3r")
            nc.vector.tensor_reduce(pack3r[:, 0:1], acc3_s[:, :8],
                                    axis=mybir.AxisListType.X, op=ADD)
            nc.vector.tensor_reduce(pack3r[:, 1:2], acc3_q[:, :8],
                                    axis=mybir.AxisListType.X, op=ADD)
            psf = pa.tile([64, 2], FP32, tag="pa")
            nc.tensor.matmul(psf[:], foldc[:], pack3r[:], start=True, stop=True)
            nc.vector.tensor_copy(packs[2][:64, :], psf[:])
            warm_keep(2, 16)
            ss3 = stats_chain(2, 64)

            # duplicate scale/shift back to the 128 pair-stacked partitions
            psd = pa.tile([128, 2], FP32, tag="pa")
            nc.tensor.matmul(psd[:], dupc[:], ss3[:, 1:3], start=True, stop=True)
            dss = statp.tile([128, 2], FP32, tag="dss")
            nc.vector.tensor_copy(dss[:], psd[:])

            # BN3 + relu -> fp32 output, quartered, split ScalarE/DVE, with
            # each quarter's output DMAs issued as soon as it lands (graph
            # 2p on the SP queue, graph 2p+1 on the ACT queue).
            for p in range(2):
                h3 = outp.tile([128, NT, 128], FP32, tag="h3")
                for hh in range(2):
                    src = agg3_cl[:, p, 8 * hh : 8 * hh + 8, :]
                    dst = h3[:, 8 * hh : 8 * hh + 8, :]
                    if hh == (p == 0):
                        nc.scalar.activation(dst, src, RELU,
                                             bias=dss[:, 1:2], scale=dss[:, 0:1])
                    else:
                        nc.vector.tensor_scalar(dst, src, dss[:, 0:1],
                                                dss[:, 1:2], MUL, ADD)
                        nc.vector.tensor_scalar_max(dst, dst, 0.0)
                    nc.sync.dma_start(
                        out_dram[2 * p, :, 1024 * hh : 1024 * hh + 1024],
                        h3[0:64, 8 * hh : 8 * hh + 8, :],
                    )
                    nc.sync.dma_start(
                        out_dram[2 * p + 1, :, 1024 * hh : 1024 * hh + 1024],
                        h3[64:128, 8 * hh : 8 * hh + 8, :],
                    )

    nc.compile()
    return nc


_NC_CACHE = {}


def get_program():
    if "nc" not in _NC_CACHE:
        _NC_CACHE["nc"] = build_program()
    return _NC_CACHE["nc"]


def host_prep(x, edge_index):
    """Build M+2I (u8, transposed+tiled), dis vectors, and Dr-scaled h0."""
    src = np.asarray(edge_index[0], np.int64)
    dst = np.asarray(edge_index[1], np.int64)
    deg = np.zeros(L, np.float32)
    np.add.at(deg, dst, 1.0)
    deg += 2.0
    dis = (deg ** -0.5).astype(np.float32)
    M = np.zeros((L, L), np.float32)
    np.add.at(M, (dst, src), 1.0)
    idx = np.arange(L)
    M[idx, idx] += 2.0
    assert M.max() < 256
    MT = np.ascontiguousarray(M.T)  # [src, dst]
    # at_pack[jb, kb, p, mj, kr, q] = MT[(4kb+kr)*128+p, (4jb+mj)*128+q]
    at_pack = np.ascontiguousarray(
        MT.reshape(4, 4, 128, 4, 4, 128).transpose(3, 0, 2, 4, 1, 5)
    ).astype(BF16)

    # x: [B, NREP, C0, L] -> [G_TOTAL, C0, L]; pre-scale by dis[node] (Dr);
    # h0 LC pack per graph pair: h0_all[p, k, gpair, c2]
    xg = np.asarray(x, np.float32).reshape(G_TOTAL, C0, L) * dis[None, None, :]
    h0_all = np.ascontiguousarray(
        xg.reshape(G_TOTAL // 2, 2 * C0, NT, 128).transpose(3, 2, 0, 1)
    ).astype(BF16)  # [128, NT, G_TOTAL//2, 2*C0]
    return at_pack, h0_all, dis


def build_in_maps(x, edge_index, W1, g1, be1, W2, g2, be2, W3, g3, be3):
    at_pack, h0_all, dis = host_prep(x, edge_index)

    w1 = np.concatenate([np.asarray(W1, np.float32)] * 2, axis=0).astype(BF16)
    w2 = np.asarray(W2, np.float32).astype(BF16)
    w3 = np.asarray(W3, np.float32).astype(BF16)
    bn = np.zeros((128, 6), np.float32)
    bn[:128, 0] = np.asarray(g1, np.float32)
    bn[:128, 1] = np.asarray(be1, np.float32)
    bn[:128, 2] = np.asarray(g2, np.float32)
    bn[:128, 3] = np.asarray(be2, np.float32)
    bn[:64, 4] = np.asarray(g3, np.float32)
    bn[:64, 5] = np.asarray(be3, np.float32)
    fold = np.zeros((128, 64), np.float32)
    fold[np.arange(128), np.arange(128) % 64] = 1.0
    dup = np.ascontiguousarray(fold.T)
    disr = np.ascontiguousarray(dis[None, :])                 # [1, L]
    disc = np.ascontiguousarray(dis.reshape(NT, 128).T)      # [128, NT]

    in_maps = []
    for c in range(N_CORES):
        # core c's graph pairs 2c, 2c+1 -> [2, 128, NT, 128]
        h0c = np.ascontiguousarray(
            h0_all[:, :, 2 * c : 2 * c + 2, :].transpose(2, 0, 1, 3)
        )
        in_maps.append(
            {
                "at": at_pack,
                "h0": h0c,
                "w1": w1,
                "w2": w2,
                "w3": w3,
                "bn": bn,
                "fold": fold,
                "dup": dup,
                "disr": disr,
                "disc": disc,
            }
        )
    return in_maps


def kernel(x, edge_index, W1, b1, g1, be1, W2, b2, g2, be2, W3, b3, g3, be3):
    in_maps = build_in_maps(x, edge_index, W1, g1, be1, W2, g2, be2, W3, g3, be3)
    nc = get_program()
    res = run_bass_kernel_spmd(nc, in_maps, core_ids=list(range(N_CORES)))
    out = np.concatenate([res.results[c]["out"] for c in range(N_CORES)], axis=0)
    return out.astype(np.float32)
